# revision 19
# baseline (speedup 1.0000x reference)
"""Trainium2 Bass kernel for nn_DPMultiheadAttention (L=2048, N=4, E=1024, H=16).

Sharding (8 cores): core d -> batch b = d//2, head-group g = d%2 (8 heads each).
QKV projections are tensor-parallel along the head dim; each core runs full
attention for its 8 heads and emits:
  - out_attn [L, E]: partial out-projection (host sums the two cores per batch, adds bo)
  - out_wT  [S, L]: sum over its 8 heads of attn^T (host sums pair, transposes, /16)

On-chip layout choices ("A orientation"):
  - scores^T tiles [s_partition, l_free]; exp via ScalarE (no max subtraction:
    scores ~ N(0,1) so exp is safe).
  - per-head colsum via a ones column appended to V (M=65 attn@V matmul).
  - attn@V produces O^T[d, l] directly; normalization by 1/colsum applied with a
    PE-broadcast of the batched reciprocal row.
  - W^T accumulated over heads in PSUM via identity-weight matmuls of the
    scaled exp tiles.
All matmul operands are bf16 (fp32 matmul is quarter-rate and pays slow
LDWEIGHTS); accumulation stays fp32 in PSUM. Host does all transposes/casts.
"""

import functools
import os
import sys

import numpy as np

if "/opt/trn_rl_repo" not in sys.path:
    sys.path.insert(0, "/opt/trn_rl_repo")

import ml_dtypes

import concourse.bass as bass
import concourse.mybir as mybir
import concourse.tile as tile
from concourse import bacc
from concourse.bass_utils import run_bass_kernel_spmd
from concourse.masks import make_identity

# Problem constants (hardcoded per harness contract)
L = 2048  # query length
S = 2048  # key length
N_BATCH = 4
E = 1024
H_TOT = 16
D = 64
F = 512           # per-core projected features (8 heads x 64)
H_DEV = 8         # heads per core
NEC = E // 128    # 8 e-chunks of 128
LC = 256          # l-chunk size
NLC = L // LC     # 8
ST = S // 128     # 16 s-tiles
SCALE = float(D) ** -0.5  # 0.125

F32 = mybir.dt.float32
BF16 = mybir.dt.bfloat16
NPBF16 = ml_dtypes.bfloat16

last_results = None  # BassKernelResults of the most recent kernel() call


def _build_program() -> bass.Bass:
    nc = bacc.Bacc(target_bir_lowering=False, debug=False)

    # DRAM I/O (per-core shapes; host feeds transposed bf16 tensors)
    qT_d = nc.dram_tensor("qT", [E, L], BF16, kind="ExternalInput")
    kT_d = nc.dram_tensor("kT", [E, S], BF16, kind="ExternalInput")
    vT_d = nc.dram_tensor("vT", [E, S], BF16, kind="ExternalInput")
    wqT_d = nc.dram_tensor("wqT", [E, F], BF16, kind="ExternalInput")
    wkT_d = nc.dram_tensor("wkT", [E, F], BF16, kind="ExternalInput")
    wvT_d = nc.dram_tensor("wvT", [E, F], BF16, kind="ExternalInput")
    woT_d = nc.dram_tensor("woT", [F, E], BF16, kind="ExternalInput")
    bq_d = nc.dram_tensor("bq", [F], F32, kind="ExternalInput")
    bk_d = nc.dram_tensor("bk", [F], F32, kind="ExternalInput")
    bv_d = nc.dram_tensor("bv", [F], F32, kind="ExternalInput")
    out_attn = nc.dram_tensor("out_attn", [L, E], F32, kind="ExternalOutput")
    out_wT = nc.dram_tensor("out_wT", [S, L], F32, kind="ExternalOutput")

    with tile.TileContext(nc) as tc:
        with tc.tile_pool(name="singles", bufs=1) as singles:
            # Constants
            ident = singles.tile([128, 128], BF16)
            make_identity(nc, ident)
            ones1 = singles.tile([1, 128], F32)
            nc.vector.memset(ones1, 1.0)
            # sel[:, h, :]: one-hot row selector — picks r_all row h in the
            # PE broadcast matmul (lhsT [8, 128] with ones on partition h)

            # Persistent weights/activations
            wq_sb = singles.tile([128, NEC, F], BF16)
            for ec in range(NEC):
                nc.sync.dma_start(
                    out=wq_sb[:, ec, :],
                    in_=wqT_d[:].rearrange("(ec p) f -> p ec f", p=128)[:, ec, :],
                )
            bqv = singles.tile([128, 4], F32)
            nc.sync.dma_start(out=bqv, in_=bq_d[:].rearrange("(ft p) -> p ft", p=128))
            bkv = singles.tile([128, 4], F32)
            nc.sync.dma_start(out=bkv, in_=bk_d[:].rearrange("(ft p) -> p ft", p=128))
            bv_bcast = singles.tile([128, F], F32)
            nc.sync.dma_start(
                out=bv_bcast,
                in_=bass.AP(tensor=bv_d[:].tensor, offset=0, ap=[[0, 128], [1, F]]),
            )

            # kT_loc[p, ft, s] : projected keys, head h at rows 64*(h%2) of ft=h//2
            kT_loc = singles.tile([128, 4, S], BF16)
            # v_aug[p, st, h, 0:64] = projected V; [..., 64] = 1.0 (colsum column)
            v_aug = singles.tile([128, ST, H_DEV, D + 1], BF16)
            nc.vector.memset(v_aug[:, :, :, D : D + 1], 1.0)

            # ---------------- K / V projections ----------------
            with (
                tc.tile_pool(name="kv_in", bufs=1) as kv_in,
                tc.tile_pool(name="kv_w", bufs=1) as kv_w,
                tc.tile_pool(name="proj_ps", bufs=2, space="PSUM") as proj_ps,
            ):
                wk_sb = kv_w.tile([128, NEC, F], BF16, tag="w")
                kin = kv_in.tile([128, NEC, S], BF16, tag="in")
                for ec in range(NEC):
                    nc.sync.dma_start(
                        out=wk_sb[:, ec, :],
                        in_=wkT_d[:].rearrange("(ec p) f -> p ec f", p=128)[:, ec, :],
                    )
                    nc.sync.dma_start(
                        out=kin[:, ec, :],
                        in_=kT_d[:].rearrange("(ec p) s -> p ec s", p=128)[:, ec, :],
                    )
                for sc in range(S // 512):
                    for ft in range(4):
                        ps = proj_ps.tile([128, 512], F32, tag="ps")
                        for ec in range(NEC):
                            nc.tensor.matmul(
                                ps,
                                wk_sb[:, ec, ft * 128 : (ft + 1) * 128],
                                kin[:, ec, sc * 512 : (sc + 1) * 512],
                                start=(ec == 0),
                                stop=(ec == NEC - 1),
                            )
                        nc.scalar.activation(
                            kT_loc[:, ft, sc * 512 : (sc + 1) * 512],
                            ps,
                            mybir.ActivationFunctionType.Identity,
                            bias=bkv[:, ft : ft + 1],
                        )

                wv_sb = kv_w.tile([128, NEC, F], BF16, tag="w")
                vin = kv_in.tile([128, NEC, S], BF16, tag="in")
                for ec in range(NEC):
                    nc.sync.dma_start(
                        out=wv_sb[:, ec, :],
                        in_=wvT_d[:].rearrange("(ec p) f -> p ec f", p=128)[:, ec, :],
                    )
                    nc.sync.dma_start(
                        out=vin[:, ec, :],
                        in_=vT_d[:].rearrange("(ec p) s -> p ec s", p=128)[:, ec, :],
                    )
                for st in range(ST):
                    ps = proj_ps.tile([128, F], F32, tag="ps")
                    for ec in range(NEC):
                        nc.tensor.matmul(
                            ps,
                            vin[:, ec, st * 128 : (st + 1) * 128],
                            wv_sb[:, ec, :],
                            start=(ec == 0),
                            stop=(ec == NEC - 1),
                        )
                    # add bias, cast to bf16, scatter into the 65-stride v_aug layout
                    nc.vector.tensor_add(
                        v_aug[:, st, :, 0:D],
                        ps.rearrange("p (h d) -> p h d", h=H_DEV),
                        bv_bcast.rearrange("p (h d) -> p h d", h=H_DEV),
                    )

            # ---------------- main attention loop ----------------
            with tc.tile_pool(name="ot", bufs=1) as ot_pool:
                # OT_all[p, ft, l]: normalized O^T (f = 4*128 rows), bf16
                ot_all = ot_pool.tile([128, 4, L], BF16)
                _main_loop(nc, tc, kT_loc, v_aug, wq_sb, bqv, ident, ones1,
                           qT_d, out_wT, ot_all)
                _out_proj(nc, tc, woT_d, out_attn, ot_all)

    nc.finalize()
    return nc


def _main_loop(nc, tc, kT_loc, v_aug, wq_sb, bqv, ident, ones1, qT_d, out_wT, ot_all):
    with (
        tc.tile_pool(name="qin", bufs=2) as qin_pool,
        tc.tile_pool(name="qt", bufs=2) as qt_pool,
        tc.tile_pool(name="exp", bufs=1) as exp_pool,
        tc.tile_pool(name="rb", bufs=2) as rb_pool,
        tc.tile_pool(name="small", bufs=4) as small_pool,
        tc.tile_pool(name="scaled", bufs=3) as scaled_pool,
        tc.tile_pool(name="wout", bufs=2) as wout_pool,
        tc.tile_pool(name="score_ps", bufs=2, space="PSUM") as score_ps,
        tc.tile_pool(name="small_ps", bufs=2, space="PSUM") as small_ps,
        tc.tile_pool(name="w_ps", bufs=2, space="PSUM") as w_ps,
    ):
        for lc in range(NLC):
            lsl = slice(lc * LC, (lc + 1) * LC)

            # q projection for this l-chunk
            qin = qin_pool.tile([128, NEC, LC], BF16, tag="qin")
            for ec in range(NEC):
                nc.sync.dma_start(
                    out=qin[:, ec, :],
                    in_=qT_d[:].rearrange("(ec p) l -> p ec l", p=128)[:, ec, lsl],
                )
            qt = qt_pool.tile([128, 4, LC], BF16, tag="qt")
            for ft in range(4):
                ps = score_ps.tile([128, 4, LC], F32, tag="sc")
                for ec in range(NEC):
                    nc.tensor.matmul(
                        ps[:, 0, :],
                        wq_sb[:, ec, ft * 128 : (ft + 1) * 128],
                        qin[:, ec, :],
                        start=(ec == 0),
                        stop=(ec == NEC - 1),
                    )
                nc.scalar.activation(
                    qt[:, ft, :],
                    ps[:, 0, :],
                    mybir.ActivationFunctionType.Identity,
                    bias=bqv[:, ft : ft + 1],
                )

            exp_sb = exp_pool.tile([128, ST, H_DEV, LC], BF16, tag="exp")
            rb_all = rb_pool.tile([128, H_DEV, LC], BF16, tag="rb")

            for hp in range(4):  # head pairs packed on PE row-groups
                # scores^T + exp, groups of 4 s-tiles
                for grp in range(4):
                    pss = [
                        score_ps.tile([128, 4, LC], F32, tag="sc", name=f"pss{i}")
                        for i in range(2)
                    ]
                    for hh in range(2):
                        prow = slice(64 * hh, 64 * (hh + 1))
                        for i in range(4):
                            st = grp * 4 + i
                            nc.tensor.matmul(
                                pss[hh][:, i, :],
                                kT_loc[prow, hp, st * 128 : (st + 1) * 128],
                                qt[prow, hp, :],
                                start=True,
                                stop=True,
                            )
                    for hh in range(2):
                        h = 2 * hp + hh
                        nc.scalar.activation(
                            exp_sb[:, grp * 4 : (grp + 1) * 4, h, :],
                            pss[hh],
                            mybir.ActivationFunctionType.Exp,
                            scale=SCALE,
                        )

                # attn @ V (+ colsum via the ones column), per head
                for hh in range(2):
                    h = 2 * hp + hh
                    pO = small_ps.tile([D + 1, LC], F32, tag="sm")
                    for st in range(ST):
                        nc.tensor.matmul(
                            pO,
                            v_aug[:, st, h, :],
                            exp_sb[:, st, h, :],
                            start=(st == 0),
                            stop=(st == ST - 1),
                        )
                    cs_sb = small_pool.tile([1, LC], F32, tag="cs")
                    nc.vector.tensor_copy(cs_sb, pO[D : D + 1, :])
                    r_sb = small_pool.tile([1, LC], F32, tag="r")
                    nc.vector.reciprocal_approx_fast(r_sb, cs_sb)
                    prb = small_ps.tile([128, LC], F32, tag="sm")
                    nc.tensor.matmul(prb, ones1, r_sb, start=True, stop=True)
                    nc.vector.tensor_copy(rb_all[:, h, :], prb)
                    # normalized O^T into OT_all rows 64*hh.. of ft=hp
                    nc.vector.tensor_mul(
                        ot_all[64 * hh : 64 * hh + 64, hp, lsl],
                        pO[0:D, :],
                        rb_all[0:D, h, :],
                    )

            # W^T accumulation over heads (identity matmuls). The scale
            # multiply is split in half-head chunks (the low heads' scales are
            # ready mid-way through the attn@V phase) and partially offloaded
            # to the otherwise-idle GPSIMD.
            for st in range(ST):
                scaled = scaled_pool.tile([128, H_DEV, LC], BF16, tag="scl")
                eng = nc.gpsimd if st % 3 == 1 else nc.vector
                eng.tensor_mul(
                    scaled[:, 0:4, :], exp_sb[:, st, 0:4, :], rb_all[:, 0:4, :]
                )
                eng.tensor_mul(
                    scaled[:, 4:8, :], exp_sb[:, st, 4:8, :], rb_all[:, 4:8, :]
                )
                pW = w_ps.tile([128, LC], F32, tag="w")
                for h in range(H_DEV):
                    nc.tensor.matmul(
                        pW,
                        ident,
                        scaled[:, h, :],
                        start=(h == 0),
                        stop=(h == H_DEV - 1),
                    )
                wo_sb = wout_pool.tile([128, LC], F32, tag="wo")
                nc.vector.tensor_copy(wo_sb, pW)
                nc.sync.dma_start(
                    out=out_wT[st * 128 : (st + 1) * 128, lsl], in_=wo_sb
                )


def _out_proj(nc, tc, woT_d, out_attn, ot_all):
    with (
        tc.tile_pool(name="late", bufs=1) as late,
        tc.tile_pool(name="ob", bufs=3) as ob_pool,
        tc.tile_pool(name="op_ps", bufs=2, space="PSUM") as op_ps,
    ):
        wo16 = late.tile([128, 4, E], BF16)
        for ft in range(4):
            nc.sync.dma_start(
                out=wo16[:, ft, :],
                in_=woT_d[:].rearrange("(ft p) e -> p ft e", p=128)[:, ft, :],
            )
        for lt in range(L // 128):
            for eo in range(E // 512):
                po = op_ps.tile([128, 512], F32, tag="op")
                for ft in range(4):
                    nc.tensor.matmul(
                        po,
                        ot_all[:, ft, lt * 128 : (lt + 1) * 128],
                        wo16[:, ft, eo * 512 : (eo + 1) * 512],
                        start=(ft == 0),
                        stop=(ft == 3),
                    )
                ob = ob_pool.tile([128, 512], F32, tag="ob")
                nc.scalar.activation(ob, po, mybir.ActivationFunctionType.Copy)
                nc.sync.dma_start(
                    out=out_attn[
                        lt * 128 : (lt + 1) * 128, eo * 512 : (eo + 1) * 512
                    ],
                    in_=ob,
                )


def _install_profile_shims():
    """Dev-only (KERNEL_TRACE=1): register the axon NTFF profile hook that the
    agent image's antenv lacks, and stub the S3 artifact upload."""
    import types

    import antenv
    import concourse.bass_utils as bu

    if "antenv.axon_hooks" not in sys.modules:
        mod = types.ModuleType("antenv.axon_hooks")
        mod._hook = None

        def set_axon_ntff_profile_hook(h):
            mod._hook = h

        def get_axon_ntff_profile_hook():
            return mod._hook

        mod.set_axon_ntff_profile_hook = set_axon_ntff_profile_hook
        mod.get_axon_ntff_profile_hook = get_axon_ntff_profile_hook
        sys.modules["antenv.axon_hooks"] = mod
        antenv.axon_hooks = mod
        from trn_agent_boot.trn_boot import _ntff_profile_via_ctypes

        mod.set_axon_ntff_profile_hook(
            _ntff_profile_via_ctypes("/opt/axon/libaxon_pjrt.so")
        )
    bu.upload_artifacts = lambda tmpdir: f"local://{tmpdir}"


@functools.lru_cache(maxsize=1)
def _get_program():
    return _build_program()


def _bf16(a):
    return np.ascontiguousarray(a).astype(NPBF16)


def kernel(query, key, value, Wq, bq, Wk, bk, Wv, bv, Wo, bo):
    query = np.asarray(query, dtype=np.float32)
    key = np.asarray(key, dtype=np.float32)
    value = np.asarray(value, dtype=np.float32)
    Wq, bq = np.asarray(Wq, np.float32), np.asarray(bq, np.float32)
    Wk, bk = np.asarray(Wk, np.float32), np.asarray(bk, np.float32)
    Wv, bv = np.asarray(Wv, np.float32), np.asarray(bv, np.float32)
    Wo, bo = np.asarray(Wo, np.float32), np.asarray(bo, np.float32)

    nc = _get_program()

    in_maps = []
    for d in range(8):
        b, g = d // 2, d % 2
        fsl = slice(F * g, F * (g + 1))
        in_maps.append(
            {
                "qT": _bf16(query[:, b, :].T),
                "kT": _bf16(key[:, b, :].T),
                "vT": _bf16(value[:, b, :].T),
                "wqT": _bf16(Wq[fsl, :].T),
                "wkT": _bf16(Wk[fsl, :].T),
                "wvT": _bf16(Wv[fsl, :].T),
                "woT": _bf16(Wo[:, fsl].T),
                "bq": np.ascontiguousarray(bq[fsl]),
                "bk": np.ascontiguousarray(bk[fsl]),
                "bv": np.ascontiguousarray(bv[fsl]),
            }
        )

    trace = bool(os.environ.get("KERNEL_TRACE"))
    tmpdir = os.environ.get("KERNEL_TRACE_DIR") or None
    if trace:
        _install_profile_shims()
    global last_results
    last_results = run_bass_kernel_spmd(
        nc, in_maps, list(range(8)), trace=trace, tmpdir=tmpdir
    )
    res = last_results.results

    attn_output = np.empty((L, N_BATCH, E), np.float32)
    avg_weights = np.empty((N_BATCH, L, S), np.float32)
    for b in range(N_BATCH):
        attn_output[:, b, :] = res[2 * b]["out_attn"] + res[2 * b + 1]["out_attn"] + bo
        avg_weights[b] = (res[2 * b]["out_wT"] + res[2 * b + 1]["out_wT"]).T / H_TOT
    return attn_output, avg_weights


# revision 21
# speedup vs baseline: 1.2528x; 1.2528x over previous
"""Trainium2 Bass kernel for nn_DPMultiheadAttention (L=2048, N=4, E=1024, H=16).

Sharding (8 cores): core d -> batch b = d//2, head-group g = d%2 (8 heads each).
QKV projections are tensor-parallel along the head dim; each core runs full
attention for its 8 heads and emits:
  - out_attn [L, E]: partial out-projection (host sums the two cores per batch, adds bo)
  - out_wT  [S, L]: sum over its 8 heads of attn^T (host sums pair, transposes, /16)

On-chip layout choices ("A orientation"):
  - scores^T tiles [s_partition, l_free]; exp via ScalarE (no max subtraction:
    scores ~ N(0,1) so exp is safe).
  - per-head colsum via a ones column appended to V (M=65 attn@V matmul).
  - attn@V produces O^T[d, l] directly; normalization by 1/colsum applied with a
    PE-broadcast of the batched reciprocal row.
  - W^T accumulated over heads in PSUM via identity-weight matmuls of the
    scaled exp tiles.
All matmul operands are bf16 (fp32 matmul is quarter-rate and pays slow
LDWEIGHTS); accumulation stays fp32 in PSUM. Host does all transposes/casts.
"""

import functools
import os
import sys

import numpy as np

if "/opt/trn_rl_repo" not in sys.path:
    sys.path.insert(0, "/opt/trn_rl_repo")

import ml_dtypes

import concourse.bass as bass
import concourse.mybir as mybir
import concourse.tile as tile
from concourse import bacc
from concourse.bass_utils import run_bass_kernel_spmd
from concourse.masks import make_identity

# Problem constants (hardcoded per harness contract)
L = 2048  # query length
S = 2048  # key length
N_BATCH = 4
E = 1024
H_TOT = 16
D = 64
F = 512           # per-core projected features (8 heads x 64)
H_DEV = 8         # heads per core
NEC = E // 128    # 8 e-chunks of 128
LC = 256          # l-chunk size
NLC = L // LC     # 8
ST = S // 128     # 16 s-tiles
SCALE = float(D) ** -0.5  # 0.125

F32 = mybir.dt.float32
BF16 = mybir.dt.bfloat16
NPBF16 = ml_dtypes.bfloat16

last_results = None  # BassKernelResults of the most recent kernel() call


def _build_program() -> bass.Bass:
    nc = bacc.Bacc(target_bir_lowering=False, debug=False)

    # DRAM I/O (per-core shapes; host feeds transposed bf16 tensors)
    qT_d = nc.dram_tensor("qT", [E, L], BF16, kind="ExternalInput")
    kT_d = nc.dram_tensor("kT", [E, S], BF16, kind="ExternalInput")
    vT_d = nc.dram_tensor("vT", [E, S], BF16, kind="ExternalInput")
    wqT_d = nc.dram_tensor("wqT", [E, F], BF16, kind="ExternalInput")
    wkT_d = nc.dram_tensor("wkT", [E, F], BF16, kind="ExternalInput")
    wvT_d = nc.dram_tensor("wvT", [E, F], BF16, kind="ExternalInput")
    woT_d = nc.dram_tensor("woT", [F, E], BF16, kind="ExternalInput")
    bq_d = nc.dram_tensor("bq", [F], F32, kind="ExternalInput")
    bk_d = nc.dram_tensor("bk", [F], F32, kind="ExternalInput")
    bv_d = nc.dram_tensor("bv", [F], F32, kind="ExternalInput")
    out_attn = nc.dram_tensor("out_attn", [L, E], F32, kind="ExternalOutput")
    out_wT = nc.dram_tensor("out_wT", [S, L], F32, kind="ExternalOutput")

    with tile.TileContext(nc) as tc:
        with tc.tile_pool(name="singles", bufs=1) as singles:
            # Constants
            ident = singles.tile([128, 128], BF16)
            make_identity(nc, ident)
            ones1 = singles.tile([1, 128], F32)
            nc.vector.memset(ones1, 1.0)
            # sel[:, h, :]: one-hot row selector — picks r_all row h in the
            # PE broadcast matmul (lhsT [8, 128] with ones on partition h)

            # Persistent weights/activations
            wq_sb = singles.tile([128, NEC, F], BF16)
            for ec in range(NEC):
                nc.sync.dma_start(
                    out=wq_sb[:, ec, :],
                    in_=wqT_d[:].rearrange("(ec p) f -> p ec f", p=128)[:, ec, :],
                )
            bqv = singles.tile([128, 4], F32)
            nc.sync.dma_start(out=bqv, in_=bq_d[:].rearrange("(ft p) -> p ft", p=128))
            bkv = singles.tile([128, 4], F32)
            nc.sync.dma_start(out=bkv, in_=bk_d[:].rearrange("(ft p) -> p ft", p=128))
            bv_bcast = singles.tile([128, F], F32)
            nc.sync.dma_start(
                out=bv_bcast,
                in_=bass.AP(tensor=bv_d[:].tensor, offset=0, ap=[[0, 128], [1, F]]),
            )

            # kT_loc[p, ft, s] : projected keys, head h at rows 64*(h%2) of ft=h//2
            kT_loc = singles.tile([128, 4, S], BF16)
            # v_aug[p, st, h, 0:64] = projected V; [..., 64] = 1.0 (colsum column)
            v_aug = singles.tile([128, ST, H_DEV, D + 1], BF16)
            nc.vector.memset(v_aug[:, :, :, D : D + 1], 1.0)

            # ---------------- K / V projections ----------------
            with (
                tc.tile_pool(name="kv_in", bufs=1) as kv_in,
                tc.tile_pool(name="kv_w", bufs=1) as kv_w,
                tc.tile_pool(name="proj_ps", bufs=2, space="PSUM") as proj_ps,
            ):
                wk_sb = kv_w.tile([128, NEC, F], BF16, tag="w")
                kin = kv_in.tile([128, NEC, S], BF16, tag="in")
                for ec in range(NEC):
                    nc.sync.dma_start(
                        out=wk_sb[:, ec, :],
                        in_=wkT_d[:].rearrange("(ec p) f -> p ec f", p=128)[:, ec, :],
                    )
                    nc.sync.dma_start(
                        out=kin[:, ec, :],
                        in_=kT_d[:].rearrange("(ec p) s -> p ec s", p=128)[:, ec, :],
                    )
                for sc in range(S // 512):
                    for ft in range(4):
                        ps = proj_ps.tile([128, 512], F32, tag="ps")
                        for ec in range(NEC):
                            nc.tensor.matmul(
                                ps,
                                wk_sb[:, ec, ft * 128 : (ft + 1) * 128],
                                kin[:, ec, sc * 512 : (sc + 1) * 512],
                                start=(ec == 0),
                                stop=(ec == NEC - 1),
                            )
                        nc.scalar.activation(
                            kT_loc[:, ft, sc * 512 : (sc + 1) * 512],
                            ps,
                            mybir.ActivationFunctionType.Identity,
                            bias=bkv[:, ft : ft + 1],
                        )

                wv_sb = kv_w.tile([128, NEC, F], BF16, tag="w")
                vin = kv_in.tile([128, NEC, S], BF16, tag="in")
                for ec in range(NEC):
                    nc.sync.dma_start(
                        out=wv_sb[:, ec, :],
                        in_=wvT_d[:].rearrange("(ec p) f -> p ec f", p=128)[:, ec, :],
                    )
                    nc.sync.dma_start(
                        out=vin[:, ec, :],
                        in_=vT_d[:].rearrange("(ec p) s -> p ec s", p=128)[:, ec, :],
                    )
                for st in range(ST):
                    ps = proj_ps.tile([128, F], F32, tag="ps")
                    for ec in range(NEC):
                        nc.tensor.matmul(
                            ps,
                            vin[:, ec, st * 128 : (st + 1) * 128],
                            wv_sb[:, ec, :],
                            start=(ec == 0),
                            stop=(ec == NEC - 1),
                        )
                    # add bias, cast to bf16, scatter into the 65-stride v_aug layout
                    nc.vector.tensor_add(
                        v_aug[:, st, :, 0:D],
                        ps.rearrange("p (h d) -> p h d", h=H_DEV),
                        bv_bcast.rearrange("p (h d) -> p h d", h=H_DEV),
                    )

            # ---------------- main attention loop ----------------
            with tc.tile_pool(name="ot", bufs=1) as ot_pool:
                # OT_all[p, ft, l]: normalized O^T (f = 4*128 rows), bf16
                ot_all = ot_pool.tile([128, 4, L], BF16)
                _main_loop(nc, tc, kT_loc, v_aug, wq_sb, bqv, ident, ones1,
                           qT_d, out_wT, ot_all)
                _out_proj(nc, tc, woT_d, out_attn, ot_all)

    nc.finalize()
    return nc


def _main_loop(nc, tc, kT_loc, v_aug, wq_sb, bqv, ident, ones1, qT_d, out_wT, ot_all):
    with (
        tc.tile_pool(name="qin", bufs=1) as qin_pool,
        tc.tile_pool(name="qt", bufs=2) as qt_pool,
        tc.tile_pool(name="exp", bufs=5) as exp_pool,
        tc.tile_pool(name="rb", bufs=2) as rb_pool,
        tc.tile_pool(name="small", bufs=4) as small_pool,
        tc.tile_pool(name="scaled", bufs=2) as scaled_pool,
        tc.tile_pool(name="wout", bufs=2) as wout_pool,
        tc.tile_pool(name="score_ps", bufs=2, space="PSUM") as score_ps,
        tc.tile_pool(name="small_ps", bufs=2, space="PSUM") as small_ps,
        tc.tile_pool(name="w_ps", bufs=2, space="PSUM") as w_ps,
    ):
        for lc in range(NLC):
            lsl = slice(lc * LC, (lc + 1) * LC)

            # q projection for this l-chunk
            qin = qin_pool.tile([128, NEC, LC], BF16, tag="qin")
            for ec in range(NEC):
                nc.sync.dma_start(
                    out=qin[:, ec, :],
                    in_=qT_d[:].rearrange("(ec p) l -> p ec l", p=128)[:, ec, lsl],
                )
            qt = qt_pool.tile([128, 4, LC], BF16, tag="qt")
            for ft in range(4):
                ps = score_ps.tile([128, 4, LC], F32, tag="sc")
                for ec in range(NEC):
                    nc.tensor.matmul(
                        ps[:, 0, :],
                        wq_sb[:, ec, ft * 128 : (ft + 1) * 128],
                        qin[:, ec, :],
                        start=(ec == 0),
                        stop=(ec == NEC - 1),
                    )
                nc.scalar.activation(
                    qt[:, ft, :],
                    ps[:, 0, :],
                    mybir.ActivationFunctionType.Identity,
                    bias=bqv[:, ft : ft + 1],
                )

            exp_q = [
                exp_pool.tile([128, 4, H_DEV, LC], BF16, tag="exp", name=f"expq{i}")
                for i in range(4)
            ]
            rb_all = rb_pool.tile([128, H_DEV, LC], BF16, tag="rb")

            for hp in range(4):  # head pairs packed on PE row-groups
                # scores^T + exp, groups of 4 s-tiles
                for grp in range(4):
                    pss = [
                        score_ps.tile([128, 4, LC], F32, tag="sc", name=f"pss{i}")
                        for i in range(2)
                    ]
                    for hh in range(2):
                        prow = slice(64 * hh, 64 * (hh + 1))
                        for i in range(4):
                            st = grp * 4 + i
                            nc.tensor.matmul(
                                pss[hh][:, i, :],
                                kT_loc[prow, hp, st * 128 : (st + 1) * 128],
                                qt[prow, hp, :],
                                start=True,
                                stop=True,
                            )
                    for hh in range(2):
                        h = 2 * hp + hh
                        nc.scalar.activation(
                            exp_q[grp][:, :, h, :],
                            pss[hh],
                            mybir.ActivationFunctionType.Exp,
                            scale=SCALE,
                        )

                # attn @ V (+ colsum via the ones column), per head
                for hh in range(2):
                    h = 2 * hp + hh
                    pO = small_ps.tile([D + 1, LC], F32, tag="sm")
                    for st in range(ST):
                        nc.tensor.matmul(
                            pO,
                            v_aug[:, st, h, :],
                            exp_q[st // 4][:, st % 4, h, :],
                            start=(st == 0),
                            stop=(st == ST - 1),
                        )
                    cs_sb = small_pool.tile([1, LC], F32, tag="cs")
                    nc.vector.tensor_copy(cs_sb, pO[D : D + 1, :])
                    r_sb = small_pool.tile([1, LC], F32, tag="r")
                    nc.vector.reciprocal_approx_fast(r_sb, cs_sb)
                    prb = small_ps.tile([128, LC], F32, tag="sm")
                    nc.tensor.matmul(prb, ones1, r_sb, start=True, stop=True)
                    nc.vector.tensor_copy(rb_all[:, h, :], prb)
                    # normalized O^T into OT_all rows 64*hh.. of ft=hp
                    nc.vector.tensor_mul(
                        ot_all[64 * hh : 64 * hh + 64, hp, lsl],
                        pO[0:D, :],
                        rb_all[0:D, h, :],
                    )

            # W^T accumulation over heads (identity matmuls)
            for st in range(ST):
                scaled = scaled_pool.tile([128, H_DEV, LC], BF16, tag="scl")
                nc.vector.tensor_mul(scaled, exp_q[st // 4][:, st % 4, :, :], rb_all)
                pW = w_ps.tile([128, LC], F32, tag="w")
                for h in range(H_DEV):
                    nc.tensor.matmul(
                        pW,
                        ident,
                        scaled[:, h, :],
                        start=(h == 0),
                        stop=(h == H_DEV - 1),
                    )
                wo_sb = wout_pool.tile([128, LC], F32, tag="wo")
                nc.vector.tensor_copy(wo_sb, pW)
                nc.sync.dma_start(
                    out=out_wT[st * 128 : (st + 1) * 128, lsl], in_=wo_sb
                )


def _out_proj(nc, tc, woT_d, out_attn, ot_all):
    with (
        tc.tile_pool(name="late", bufs=1) as late,
        tc.tile_pool(name="ob", bufs=3) as ob_pool,
        tc.tile_pool(name="op_ps", bufs=2, space="PSUM") as op_ps,
    ):
        wo16 = late.tile([128, 4, E], BF16)
        for ft in range(4):
            nc.sync.dma_start(
                out=wo16[:, ft, :],
                in_=woT_d[:].rearrange("(ft p) e -> p ft e", p=128)[:, ft, :],
            )
        for lt in range(L // 128):
            for eo in range(E // 512):
                po = op_ps.tile([128, 512], F32, tag="op")
                for ft in range(4):
                    nc.tensor.matmul(
                        po,
                        ot_all[:, ft, lt * 128 : (lt + 1) * 128],
                        wo16[:, ft, eo * 512 : (eo + 1) * 512],
                        start=(ft == 0),
                        stop=(ft == 3),
                    )
                ob = ob_pool.tile([128, 512], F32, tag="ob")
                nc.scalar.activation(ob, po, mybir.ActivationFunctionType.Copy)
                nc.sync.dma_start(
                    out=out_attn[
                        lt * 128 : (lt + 1) * 128, eo * 512 : (eo + 1) * 512
                    ],
                    in_=ob,
                )


def _install_profile_shims():
    """Dev-only (KERNEL_TRACE=1): register the axon NTFF profile hook that the
    agent image's antenv lacks, and stub the S3 artifact upload."""
    import types

    import antenv
    import concourse.bass_utils as bu

    if "antenv.axon_hooks" not in sys.modules:
        mod = types.ModuleType("antenv.axon_hooks")
        mod._hook = None

        def set_axon_ntff_profile_hook(h):
            mod._hook = h

        def get_axon_ntff_profile_hook():
            return mod._hook

        mod.set_axon_ntff_profile_hook = set_axon_ntff_profile_hook
        mod.get_axon_ntff_profile_hook = get_axon_ntff_profile_hook
        sys.modules["antenv.axon_hooks"] = mod
        antenv.axon_hooks = mod
        from trn_agent_boot.trn_boot import _ntff_profile_via_ctypes

        mod.set_axon_ntff_profile_hook(
            _ntff_profile_via_ctypes("/opt/axon/libaxon_pjrt.so")
        )
    bu.upload_artifacts = lambda tmpdir: f"local://{tmpdir}"


@functools.lru_cache(maxsize=1)
def _get_program():
    return _build_program()


def _bf16(a):
    return np.ascontiguousarray(a).astype(NPBF16)


def kernel(query, key, value, Wq, bq, Wk, bk, Wv, bv, Wo, bo):
    query = np.asarray(query, dtype=np.float32)
    key = np.asarray(key, dtype=np.float32)
    value = np.asarray(value, dtype=np.float32)
    Wq, bq = np.asarray(Wq, np.float32), np.asarray(bq, np.float32)
    Wk, bk = np.asarray(Wk, np.float32), np.asarray(bk, np.float32)
    Wv, bv = np.asarray(Wv, np.float32), np.asarray(bv, np.float32)
    Wo, bo = np.asarray(Wo, np.float32), np.asarray(bo, np.float32)

    nc = _get_program()

    in_maps = []
    for d in range(8):
        b, g = d // 2, d % 2
        fsl = slice(F * g, F * (g + 1))
        in_maps.append(
            {
                "qT": _bf16(query[:, b, :].T),
                "kT": _bf16(key[:, b, :].T),
                "vT": _bf16(value[:, b, :].T),
                "wqT": _bf16(Wq[fsl, :].T),
                "wkT": _bf16(Wk[fsl, :].T),
                "wvT": _bf16(Wv[fsl, :].T),
                "woT": _bf16(Wo[:, fsl].T),
                "bq": np.ascontiguousarray(bq[fsl]),
                "bk": np.ascontiguousarray(bk[fsl]),
                "bv": np.ascontiguousarray(bv[fsl]),
            }
        )

    trace = bool(os.environ.get("KERNEL_TRACE"))
    tmpdir = os.environ.get("KERNEL_TRACE_DIR") or None
    if trace:
        _install_profile_shims()
    global last_results
    last_results = run_bass_kernel_spmd(
        nc, in_maps, list(range(8)), trace=trace, tmpdir=tmpdir
    )
    res = last_results.results

    attn_output = np.empty((L, N_BATCH, E), np.float32)
    avg_weights = np.empty((N_BATCH, L, S), np.float32)
    for b in range(N_BATCH):
        attn_output[:, b, :] = res[2 * b]["out_attn"] + res[2 * b + 1]["out_attn"] + bo
        avg_weights[b] = (res[2 * b]["out_wT"] + res[2 * b + 1]["out_wT"]).T / H_TOT
    return attn_output, avg_weights


# revision 22
# speedup vs baseline: 1.3010x; 1.0384x over previous
"""Trainium2 Bass kernel for nn_DPMultiheadAttention (L=2048, N=4, E=1024, H=16).

Sharding (8 cores): core d -> batch b = d//2, head-group g = d%2 (8 heads each).
QKV projections are tensor-parallel along the head dim; each core runs full
attention for its 8 heads and emits:
  - out_attn [L, E]: partial out-projection (host sums the two cores per batch, adds bo)
  - out_wT  [S, L]: sum over its 8 heads of attn^T (host sums pair, transposes, /16)

On-chip layout choices ("A orientation"):
  - scores^T tiles [s_partition, l_free]; exp via ScalarE (no max subtraction:
    scores ~ N(0,1) so exp is safe).
  - per-head colsum via a ones column appended to V (M=65 attn@V matmul).
  - attn@V produces O^T[d, l] directly; normalization by 1/colsum applied with a
    PE-broadcast of the batched reciprocal row.
  - W^T accumulated over heads in PSUM via identity-weight matmuls of the
    scaled exp tiles.
All matmul operands are bf16 (fp32 matmul is quarter-rate and pays slow
LDWEIGHTS); accumulation stays fp32 in PSUM. Host does all transposes/casts.
"""

import functools
import os
import sys

import numpy as np

if "/opt/trn_rl_repo" not in sys.path:
    sys.path.insert(0, "/opt/trn_rl_repo")

import ml_dtypes

import concourse.bass as bass
import concourse.mybir as mybir
import concourse.tile as tile
from concourse import bacc
from concourse.bass_utils import run_bass_kernel_spmd
from concourse.masks import make_identity

# Problem constants (hardcoded per harness contract)
L = 2048  # query length
S = 2048  # key length
N_BATCH = 4
E = 1024
H_TOT = 16
D = 64
F = 512           # per-core projected features (8 heads x 64)
H_DEV = 8         # heads per core
NEC = E // 128    # 8 e-chunks of 128
LC = 256          # l-chunk size
NLC = L // LC     # 8
ST = S // 128     # 16 s-tiles
SCALE = float(D) ** -0.5  # 0.125

F32 = mybir.dt.float32
BF16 = mybir.dt.bfloat16
NPBF16 = ml_dtypes.bfloat16

last_results = None  # BassKernelResults of the most recent kernel() call


def _build_program() -> bass.Bass:
    nc = bacc.Bacc(target_bir_lowering=False, debug=False)

    # DRAM I/O (per-core shapes; host feeds transposed bf16 tensors)
    qT_d = nc.dram_tensor("qT", [E, L], BF16, kind="ExternalInput")
    kT_d = nc.dram_tensor("kT", [E, S], BF16, kind="ExternalInput")
    vT_d = nc.dram_tensor("vT", [E, S], BF16, kind="ExternalInput")
    wqT_d = nc.dram_tensor("wqT", [E, F], BF16, kind="ExternalInput")
    wkT_d = nc.dram_tensor("wkT", [E, F], BF16, kind="ExternalInput")
    wvT_d = nc.dram_tensor("wvT", [E, F], BF16, kind="ExternalInput")
    woT_d = nc.dram_tensor("woT", [F, E], BF16, kind="ExternalInput")
    bq_d = nc.dram_tensor("bq", [F], F32, kind="ExternalInput")
    bk_d = nc.dram_tensor("bk", [F], F32, kind="ExternalInput")
    bv_d = nc.dram_tensor("bv", [F], F32, kind="ExternalInput")
    out_attn = nc.dram_tensor("out_attn", [L, E], F32, kind="ExternalOutput")
    out_wT = nc.dram_tensor("out_wT", [S, L], F32, kind="ExternalOutput")

    with tile.TileContext(nc) as tc:
        with tc.tile_pool(name="singles", bufs=1) as singles:
            # Constants
            ident = singles.tile([128, 128], BF16)
            make_identity(nc, ident)
            ones1 = singles.tile([1, 128], F32)
            nc.vector.memset(ones1, 1.0)
            # sel[:, h, :]: one-hot row selector — picks r_all row h in the
            # PE broadcast matmul (lhsT [8, 128] with ones on partition h)

            # Persistent weights/activations
            wq_sb = singles.tile([128, NEC, F], BF16)
            for ec in range(NEC):
                nc.sync.dma_start(
                    out=wq_sb[:, ec, :],
                    in_=wqT_d[:].rearrange("(ec p) f -> p ec f", p=128)[:, ec, :],
                )
            bqv = singles.tile([128, 4], F32)
            nc.sync.dma_start(out=bqv, in_=bq_d[:].rearrange("(ft p) -> p ft", p=128))
            bkv = singles.tile([128, 4], F32)
            nc.sync.dma_start(out=bkv, in_=bk_d[:].rearrange("(ft p) -> p ft", p=128))
            bv_bcast = singles.tile([128, F], F32)
            nc.sync.dma_start(
                out=bv_bcast,
                in_=bass.AP(tensor=bv_d[:].tensor, offset=0, ap=[[0, 128], [1, F]]),
            )

            wo16 = singles.tile([128, 4, E], BF16)
            for ft in range(4):
                nc.sync.dma_start(
                    out=wo16[:, ft, :],
                    in_=woT_d[:].rearrange("(ft p) e -> p ft e", p=128)[:, ft, :],
                )

            # kT_loc[p, ft, s] : projected keys, head h at rows 64*(h%2) of ft=h//2
            kT_loc = singles.tile([128, 4, S], BF16)
            # v_aug[p, st, h, 0:64] = projected V; [..., 64] = 1.0 (colsum column)
            v_aug = singles.tile([128, ST, H_DEV, D + 1], BF16)
            nc.vector.memset(v_aug[:, :, :, D : D + 1], 1.0)

            # ---------------- K / V projections ----------------
            with (
                tc.tile_pool(name="kv_in", bufs=1) as kv_in,
                tc.tile_pool(name="kv_w", bufs=1) as kv_w,
                tc.tile_pool(name="proj_ps", bufs=2, space="PSUM") as proj_ps,
            ):
                wk_sb = kv_w.tile([128, NEC, F], BF16, tag="w")
                kin = kv_in.tile([128, NEC, S], BF16, tag="in")
                for ec in range(NEC):
                    nc.sync.dma_start(
                        out=wk_sb[:, ec, :],
                        in_=wkT_d[:].rearrange("(ec p) f -> p ec f", p=128)[:, ec, :],
                    )
                    nc.sync.dma_start(
                        out=kin[:, ec, :],
                        in_=kT_d[:].rearrange("(ec p) s -> p ec s", p=128)[:, ec, :],
                    )
                for sc in range(S // 512):
                    for ft in range(4):
                        ps = proj_ps.tile([128, 512], F32, tag="ps")
                        for ec in range(NEC):
                            nc.tensor.matmul(
                                ps,
                                wk_sb[:, ec, ft * 128 : (ft + 1) * 128],
                                kin[:, ec, sc * 512 : (sc + 1) * 512],
                                start=(ec == 0),
                                stop=(ec == NEC - 1),
                            )
                        nc.scalar.activation(
                            kT_loc[:, ft, sc * 512 : (sc + 1) * 512],
                            ps,
                            mybir.ActivationFunctionType.Identity,
                            bias=bkv[:, ft : ft + 1],
                        )

                wv_sb = kv_w.tile([128, NEC, F], BF16, tag="w")
                vin = kv_in.tile([128, NEC, S], BF16, tag="in")
                for ec in range(NEC):
                    nc.sync.dma_start(
                        out=wv_sb[:, ec, :],
                        in_=wvT_d[:].rearrange("(ec p) f -> p ec f", p=128)[:, ec, :],
                    )
                    nc.sync.dma_start(
                        out=vin[:, ec, :],
                        in_=vT_d[:].rearrange("(ec p) s -> p ec s", p=128)[:, ec, :],
                    )
                for st in range(ST):
                    ps = proj_ps.tile([128, F], F32, tag="ps")
                    for ec in range(NEC):
                        nc.tensor.matmul(
                            ps,
                            vin[:, ec, st * 128 : (st + 1) * 128],
                            wv_sb[:, ec, :],
                            start=(ec == 0),
                            stop=(ec == NEC - 1),
                        )
                    # add bias, cast to bf16, scatter into the 65-stride v_aug layout
                    nc.vector.tensor_add(
                        v_aug[:, st, :, 0:D],
                        ps.rearrange("p (h d) -> p h d", h=H_DEV),
                        bv_bcast.rearrange("p (h d) -> p h d", h=H_DEV),
                    )

            # ---------------- main attention loop ----------------
            with tc.tile_pool(name="ot", bufs=1) as ot_pool:
                # OT_all[p, ft, l]: normalized O^T (f = 4*128 rows), bf16
                ot_all = ot_pool.tile([128, 4, L], BF16)
                _main_loop(nc, tc, kT_loc, v_aug, wq_sb, bqv, ident, ones1,
                           qT_d, out_wT, ot_all, wo16, out_attn)

    nc.finalize()
    return nc


def _main_loop(nc, tc, kT_loc, v_aug, wq_sb, bqv, ident, ones1, qT_d, out_wT, ot_all, wo16, out_attn):
    with (
        tc.tile_pool(name="qin", bufs=1) as qin_pool,
        tc.tile_pool(name="qt", bufs=2) as qt_pool,
        tc.tile_pool(name="exp", bufs=5) as exp_pool,
        tc.tile_pool(name="rb", bufs=2) as rb_pool,
        tc.tile_pool(name="small", bufs=4) as small_pool,
        tc.tile_pool(name="scaled", bufs=2) as scaled_pool,
        tc.tile_pool(name="wout", bufs=2) as wout_pool,
        tc.tile_pool(name="score_ps", bufs=2, space="PSUM") as score_ps,
        tc.tile_pool(name="small_ps", bufs=2, space="PSUM") as small_ps,
        tc.tile_pool(name="w_ps", bufs=1, space="PSUM") as w_ps,
        tc.tile_pool(name="op_ps", bufs=1, space="PSUM") as op_ps,
        tc.tile_pool(name="ob", bufs=2) as ob_pool,
    ):
        for lc in range(NLC):
            lsl = slice(lc * LC, (lc + 1) * LC)

            # q projection for this l-chunk
            qin = qin_pool.tile([128, NEC, LC], BF16, tag="qin")
            for ec in range(NEC):
                nc.sync.dma_start(
                    out=qin[:, ec, :],
                    in_=qT_d[:].rearrange("(ec p) l -> p ec l", p=128)[:, ec, lsl],
                )
            qt = qt_pool.tile([128, 4, LC], BF16, tag="qt")
            for ft in range(4):
                ps = score_ps.tile([128, 4, LC], F32, tag="sc")
                for ec in range(NEC):
                    nc.tensor.matmul(
                        ps[:, 0, :],
                        wq_sb[:, ec, ft * 128 : (ft + 1) * 128],
                        qin[:, ec, :],
                        start=(ec == 0),
                        stop=(ec == NEC - 1),
                    )
                nc.scalar.activation(
                    qt[:, ft, :],
                    ps[:, 0, :],
                    mybir.ActivationFunctionType.Identity,
                    bias=bqv[:, ft : ft + 1],
                )

            exp_q = [
                exp_pool.tile([128, 4, H_DEV, LC], BF16, tag="exp", name=f"expq{i}")
                for i in range(4)
            ]
            rb_all = rb_pool.tile([128, H_DEV, LC], BF16, tag="rb")

            for hp in range(4):  # head pairs packed on PE row-groups
                # scores^T + exp, groups of 4 s-tiles
                for grp in range(4):
                    pss = [
                        score_ps.tile([128, 4, LC], F32, tag="sc", name=f"pss{i}")
                        for i in range(2)
                    ]
                    for hh in range(2):
                        prow = slice(64 * hh, 64 * (hh + 1))
                        for i in range(4):
                            st = grp * 4 + i
                            nc.tensor.matmul(
                                pss[hh][:, i, :],
                                kT_loc[prow, hp, st * 128 : (st + 1) * 128],
                                qt[prow, hp, :],
                                start=True,
                                stop=True,
                            )
                    for hh in range(2):
                        h = 2 * hp + hh
                        nc.scalar.activation(
                            exp_q[grp][:, :, h, :],
                            pss[hh],
                            mybir.ActivationFunctionType.Exp,
                            scale=SCALE,
                        )

                # attn @ V (+ colsum via the ones column), per head
                for hh in range(2):
                    h = 2 * hp + hh
                    pO = small_ps.tile([D + 1, LC], F32, tag="sm")
                    for st in range(ST):
                        nc.tensor.matmul(
                            pO,
                            v_aug[:, st, h, :],
                            exp_q[st // 4][:, st % 4, h, :],
                            start=(st == 0),
                            stop=(st == ST - 1),
                        )
                    cs_sb = small_pool.tile([1, LC], F32, tag="cs")
                    nc.vector.tensor_copy(cs_sb, pO[D : D + 1, :])
                    r_sb = small_pool.tile([1, LC], F32, tag="r")
                    nc.vector.reciprocal_approx_fast(r_sb, cs_sb)
                    prb = small_ps.tile([128, LC], F32, tag="sm")
                    nc.tensor.matmul(prb, ones1, r_sb, start=True, stop=True)
                    nc.vector.tensor_copy(rb_all[:, h, :], prb)
                    # normalized O^T into OT_all rows 64*hh.. of ft=hp
                    nc.vector.tensor_mul(
                        ot_all[64 * hh : 64 * hh + 64, hp, lsl],
                        pO[0:D, :],
                        rb_all[0:D, h, :],
                    )

            # W^T accumulation over heads (identity matmuls)
            for st in range(ST):
                scaled = scaled_pool.tile([128, H_DEV, LC], BF16, tag="scl")
                nc.vector.tensor_mul(scaled, exp_q[st // 4][:, st % 4, :, :], rb_all)
                pW = w_ps.tile([128, LC], F32, tag="w")
                for h in range(H_DEV):
                    nc.tensor.matmul(
                        pW,
                        ident,
                        scaled[:, h, :],
                        start=(h == 0),
                        stop=(h == H_DEV - 1),
                    )
                if st % 2 == 0:
                    wo_sb = wout_pool.tile([128, 2, LC], F32, tag="wo")
                nc.vector.tensor_copy(wo_sb[:, st % 2, :], pW)
                if st % 2 == 1:
                    nc.sync.dma_start(
                        out=out_wT[:, lsl].rearrange(
                            "(g p) l -> p g l", p=128
                        )[:, st - 1 : st + 1, :],
                        in_=wo_sb,
                    )

            # out-projection for the two finished 128-row l-tiles of this chunk
            for lt in (2 * lc, 2 * lc + 1):
                for eo in range(E // 512):
                    po = op_ps.tile([128, 512], F32, tag="op")
                    for ft in range(4):
                        nc.tensor.matmul(
                            po,
                            ot_all[:, ft, lt * 128 : (lt + 1) * 128],
                            wo16[:, ft, eo * 512 : (eo + 1) * 512],
                            start=(ft == 0),
                            stop=(ft == 3),
                        )
                    ob = ob_pool.tile([128, 512], F32, tag="ob")
                    nc.scalar.activation(ob, po, mybir.ActivationFunctionType.Copy)
                    nc.sync.dma_start(
                        out=out_attn[
                            lt * 128 : (lt + 1) * 128, eo * 512 : (eo + 1) * 512
                        ],
                        in_=ob,
                    )


def _out_proj(nc, tc, woT_d, out_attn, ot_all):
    with (
        tc.tile_pool(name="late", bufs=1) as late,
        tc.tile_pool(name="ob", bufs=3) as ob_pool,
        tc.tile_pool(name="op_ps", bufs=2, space="PSUM") as op_ps,
    ):
        wo16 = late.tile([128, 4, E], BF16)
        for ft in range(4):
            nc.sync.dma_start(
                out=wo16[:, ft, :],
                in_=woT_d[:].rearrange("(ft p) e -> p ft e", p=128)[:, ft, :],
            )
        for lt in range(L // 128):
            for eo in range(E // 512):
                po = op_ps.tile([128, 512], F32, tag="op")
                for ft in range(4):
                    nc.tensor.matmul(
                        po,
                        ot_all[:, ft, lt * 128 : (lt + 1) * 128],
                        wo16[:, ft, eo * 512 : (eo + 1) * 512],
                        start=(ft == 0),
                        stop=(ft == 3),
                    )
                ob = ob_pool.tile([128, 512], F32, tag="ob")
                nc.scalar.activation(ob, po, mybir.ActivationFunctionType.Copy)
                nc.sync.dma_start(
                    out=out_attn[
                        lt * 128 : (lt + 1) * 128, eo * 512 : (eo + 1) * 512
                    ],
                    in_=ob,
                )


def _install_profile_shims():
    """Dev-only (KERNEL_TRACE=1): register the axon NTFF profile hook that the
    agent image's antenv lacks, and stub the S3 artifact upload."""
    import types

    import antenv
    import concourse.bass_utils as bu

    if "antenv.axon_hooks" not in sys.modules:
        mod = types.ModuleType("antenv.axon_hooks")
        mod._hook = None

        def set_axon_ntff_profile_hook(h):
            mod._hook = h

        def get_axon_ntff_profile_hook():
            return mod._hook

        mod.set_axon_ntff_profile_hook = set_axon_ntff_profile_hook
        mod.get_axon_ntff_profile_hook = get_axon_ntff_profile_hook
        sys.modules["antenv.axon_hooks"] = mod
        antenv.axon_hooks = mod
        from trn_agent_boot.trn_boot import _ntff_profile_via_ctypes

        mod.set_axon_ntff_profile_hook(
            _ntff_profile_via_ctypes("/opt/axon/libaxon_pjrt.so")
        )
    bu.upload_artifacts = lambda tmpdir: f"local://{tmpdir}"


@functools.lru_cache(maxsize=1)
def _get_program():
    return _build_program()


def _bf16(a):
    return np.ascontiguousarray(a).astype(NPBF16)


def kernel(query, key, value, Wq, bq, Wk, bk, Wv, bv, Wo, bo):
    query = np.asarray(query, dtype=np.float32)
    key = np.asarray(key, dtype=np.float32)
    value = np.asarray(value, dtype=np.float32)
    Wq, bq = np.asarray(Wq, np.float32), np.asarray(bq, np.float32)
    Wk, bk = np.asarray(Wk, np.float32), np.asarray(bk, np.float32)
    Wv, bv = np.asarray(Wv, np.float32), np.asarray(bv, np.float32)
    Wo, bo = np.asarray(Wo, np.float32), np.asarray(bo, np.float32)

    nc = _get_program()

    in_maps = []
    for d in range(8):
        b, g = d // 2, d % 2
        fsl = slice(F * g, F * (g + 1))
        in_maps.append(
            {
                "qT": _bf16(query[:, b, :].T),
                "kT": _bf16(key[:, b, :].T),
                "vT": _bf16(value[:, b, :].T),
                "wqT": _bf16(Wq[fsl, :].T),
                "wkT": _bf16(Wk[fsl, :].T),
                "wvT": _bf16(Wv[fsl, :].T),
                "woT": _bf16(Wo[:, fsl].T),
                "bq": np.ascontiguousarray(bq[fsl]),
                "bk": np.ascontiguousarray(bk[fsl]),
                "bv": np.ascontiguousarray(bv[fsl]),
            }
        )

    trace = bool(os.environ.get("KERNEL_TRACE"))
    tmpdir = os.environ.get("KERNEL_TRACE_DIR") or None
    if trace:
        _install_profile_shims()
    global last_results
    last_results = run_bass_kernel_spmd(
        nc, in_maps, list(range(8)), trace=trace, tmpdir=tmpdir
    )
    res = last_results.results

    attn_output = np.empty((L, N_BATCH, E), np.float32)
    avg_weights = np.empty((N_BATCH, L, S), np.float32)
    for b in range(N_BATCH):
        attn_output[:, b, :] = res[2 * b]["out_attn"] + res[2 * b + 1]["out_attn"] + bo
        avg_weights[b] = (res[2 * b]["out_wT"] + res[2 * b + 1]["out_wT"]).T / H_TOT
    return attn_output, avg_weights
